# revision 1
# baseline (speedup 1.0000x reference)
"""MoE (top-2 routed GluMLP) Trainium2 kernel, expert-parallel over 8 NeuronCores.

Contract: kernel(**inputs) takes the FULL unsharded inputs
  x  [2, 2048, 1024] f32
  Wr [8, 1024] f32           router
  Wg [8, 4096, 1024] f32     gate proj per expert
  Wu [8, 4096, 1024] f32     up proj per expert
  Wd [8, 1024, 4096] f32     down proj per expert
and returns the FULL output [2, 2048, 1024] f32.

Strategy (expert-parallel, sparse dispatch):
  - Routing (softmax + top-2 + renormalize) is computed on host with jax on
    CPU using the exact ops of the reference, so the selected expert set and
    combine weights match the reference bit-for-bit.
  - Tokens are gathered per expert (capacity = max expert load, rounded to 8)
    and dispatched to the core owning that expert.
  - Each core runs a weighted GluMLP over its Tc tokens:
        out[t, :] = w[t] * (relu(x_t @ Wg_e.T) * (x_t @ Wu_e.T)) @ Wd_e.T
    with matmul operands in fp16 (same 10-bit mantissa as TF32) and fp32
    accumulation in PSUM; host converts operands so rounding is exact.
  - Host scatter-adds the per-core outputs back into the full [T, D] output.

Env: MOE_MM_DT selects matmul operand dtype:
  f16 (default): fp16 operands (same 10-bit mantissa as TF32), fp32 accumulate.
                 Fast weight load + half DMA. ~5e-4 rel err.
  f32r:          TF32. Same accuracy, slower weight loads, 2x DMA.
  f32:           plain fp32 matmuls (4x slower), ~1e-6 rel err.
"""

import math
import os
from contextlib import ExitStack

import numpy as np

import concourse.bass as bass
import concourse.tile as tile
from concourse import bacc, mybir
from concourse.bass_utils import run_bass_kernel_spmd

B, L, D, E, TOPK, DFF = 2, 2048, 1024, 8, 2, 4096
T = B * L
NCORES = 8
P = 128
NB = 512          # matmul moving-operand block (fp32 max; one PSUM bank of fp32 out)
DC = D // P       # 8 contraction chunks over D
FM = DFF // P     # 32 chunks over DFF

F32 = mybir.dt.float32
F32R = mybir.dt.float32r
F16 = mybir.dt.float16

# Set to True (e.g. from test.py) to run with NTFF tracing and print HW time.
PROFILE = False
TRACE_CORES = None  # e.g. list(range(8)) to profile every core
LAST_EXEC_NS = None
# Matmul dtype for the big GluMLP matmuls.
MM_DT = {"f32": F32, "f32r": F32R, "f16": F16}[os.environ.get("MOE_MM_DT", "f16")]
# Token chunk held in SBUF (h_all is [128, 32, TCH] in MM_DT).
TCH = 2048 if MM_DT is F16 else 1024


def _nblocks(tch):
    """Moving-dim blocks <=512, balanced: per-matmul cost is
    max(stream_cols/2.4GHz, ~100ns weight-load floor), so equal blocks beat
    512s-plus-tiny-tail (a tail below ~233 cols is pure LDW overhead)."""
    k = max(1, math.ceil(tch / NB))
    base, rem = divmod(tch, k)
    out, t = [], 0
    for i in range(k):
        nb = base + (1 if i < rem else 0)
        out.append((t, nb))
        t += nb
    return out


def _mgroups(ntile, gmax=8):
    """Token-subtile groups of up to gmax (PSUM-bank limited), balanced so no
    group is tiny (a group of 1 can't hide its Wd stream behind matmuls)."""
    ngroups = max(1, math.ceil(ntile / gmax))
    base, rem = divmod(ntile, ngroups)
    out, m = [], 0
    for i in range(ngroups):
        g = base + (1 if i < rem else 0)
        out.append((m, g))
        m += g
    return out


def _build_nc(Tc: int):
    """Build the single-core Bass program (SPMD: all cores run the same NEFF)."""
    nc = bacc.Bacc(
        "TRN2",
        target_bir_lowering=False,
        debug=False,
        enable_asserts=False,
        num_devices=NCORES,
    )
    mdt = MM_DT
    x_d = nc.dram_tensor("x", [P, DC, Tc], mdt, kind="ExternalInput").ap()
    w_d = nc.dram_tensor("w", [math.ceil(Tc / P), P], F32, kind="ExternalInput").ap()
    wg_d = nc.dram_tensor("wg", [FM, P, DC, P], mdt, kind="ExternalInput").ap()
    wu_d = nc.dram_tensor("wu", [FM, P, DC, P], mdt, kind="ExternalInput").ap()
    wd_d = nc.dram_tensor("wd", [FM, P, D], mdt, kind="ExternalInput").ap()
    out_d = nc.dram_tensor("out", [Tc, D], F32, kind="ExternalOutput").ap()

    with tile.TileContext(nc) as tc:
        with ExitStack() as ctx:
            _moe_body(ctx, tc, x_d, w_d, wg_d, wu_d, wd_d, out_d, Tc)
    nc.compile()
    return nc


def _moe_body(ctx, tc, x_d, w_d, wg_d, wu_d, wd_d, out_d, Tc):
    nc = tc.nc
    mdt = MM_DT
    nchunk = math.ceil(Tc / TCH)
    tca = min(TCH, Tc)  # allocated chunk width (don't waste SBUF below the cap)

    xpool = ctx.enter_context(tc.tile_pool(name="xpool", bufs=1))
    hpool = ctx.enter_context(tc.tile_pool(name="hpool", bufs=1))
    wgupool = ctx.enter_context(tc.tile_pool(name="wgupool", bufs=2))
    wdpool = ctx.enter_context(tc.tile_pool(name="wdpool", bufs=6))
    gpool = ctx.enter_context(tc.tile_pool(name="gpool", bufs=3))
    opool = ctx.enter_context(tc.tile_pool(name="opool", bufs=6))
    wtpool = ctx.enter_context(tc.tile_pool(name="wtpool", bufs=2))
    # One shared PSUM pool: phase B (ps_g/ps_u) and phase C (output groups)
    # don't overlap in time, so both get all 8 banks.
    psP = ctx.enter_context(tc.tile_pool(name="psP", bufs=8, space="PSUM"))

    for ci in range(nchunk):
        t0 = ci * TCH
        tch = min(TCH, Tc - t0)
        ntile = math.ceil(tch / P)   # token subtiles of <=128 (last may be partial)

        # Prefetch the first weight tiles ahead of the x stream so phase B can
        # start the moment the last x stripe lands (they otherwise queue
        # behind the x slices on the same DMA queues).
        pre = []
        for fm in range(2):
            wg_p = wgupool.tile([P, DC, P], mdt, tag="wg", name=f"wg_pre{fm}")
            nc.sync.dma_start(out=wg_p, in_=wg_d[fm])
            wu_p = wgupool.tile([P, DC, P], mdt, tag="wu", name=f"wu_pre{fm}")
            nc.scalar.dma_start(out=wu_p, in_=wu_d[fm])
            pre.append((wg_p, wu_p))

        # x load split across the three DMA-capable engines (sync/scalar HWDGE
        # + gpsimd SWDGE) — a single queue tops out well below HBM rate.
        # (Measured: shifting more stripes onto gpsimd's SWDGE queue to
        # "balance" bytes is a net loss — SWDGE is slower per byte.)
        x_sb = xpool.tile([P, DC, tca], mdt, tag="x")
        dma_engines = [nc.sync, nc.scalar, nc.gpsimd]
        for dc in range(DC):
            eng = dma_engines[dc % 3]
            eng.dma_start(out=x_sb[:, dc, :tch], in_=x_d[:, dc, t0 : t0 + tch])
        w_sb = wtpool.tile([P, math.ceil(tca / P)], F32, tag="w")
        nc.gpsimd.dma_start(
            out=w_sb[:, :ntile],
            in_=w_d[t0 // P : t0 // P + ntile, :].rearrange("n p -> p n"),
        )

        h_all = hpool.tile([P, FM, tca], mdt, tag="h")

        # Phase B: h[f, t] = relu(g) * u for this token chunk, f-major layout.
        for fm in range(FM):
            if fm < len(pre):
                wg_sb, wu_sb = pre[fm]
            else:
                wg_sb = wgupool.tile([P, DC, P], mdt, tag="wg")
                nc.sync.dma_start(out=wg_sb, in_=wg_d[fm])
                wu_sb = wgupool.tile([P, DC, P], mdt, tag="wu")
                nc.scalar.dma_start(out=wu_sb, in_=wu_d[fm])
            for nb0, nbl in _nblocks(tch):
                ts = slice(nb0, nb0 + nbl)
                ps_g = psP.tile([P, NB], F32, tag="ps")
                ps_u = psP.tile([P, NB], F32, tag="ps")
                for dc in range(DC):
                    nc.tensor.matmul(
                        ps_g[:, :nbl],
                        lhsT=wg_sb[:, dc, :],
                        rhs=x_sb[:, dc, ts],
                        start=(dc == 0),
                        stop=(dc == DC - 1),
                    )
                for dc in range(DC):
                    nc.tensor.matmul(
                        ps_u[:, :nbl],
                        lhsT=wu_sb[:, dc, :],
                        rhs=x_sb[:, dc, ts],
                        start=(dc == 0),
                        stop=(dc == DC - 1),
                    )
                g_sb = gpool.tile([P, NB], F32, tag="g")
                nc.scalar.activation(
                    out=g_sb[:, :nbl],
                    in_=ps_g[:, :nbl],
                    func=mybir.ActivationFunctionType.Relu,
                )
                nc.vector.tensor_mul(h_all[:, fm, ts], g_sb[:, :nbl], ps_u[:, :nbl])

        # Phase C: out[t, :] = w[t] * (h.T @ WdT) for this chunk.
        # Loop dn (D half) / token groups of <=8 / fc-pairs so each Wd tile is
        # loaded once per token group (2 full Wd passes per chunk).
        for dn in range(D // NB):
            ds = slice(dn * NB, (dn + 1) * NB)
            for mg0, mgl in _mgroups(ntile):
                ps_os = []
                for j in range(mgl):
                    ps_o = psP.tile([P, NB], F32, tag="ps", name=f"ps_o{j}")
                    ps_os.append(ps_o)
                for fc2 in range(FM // 2):
                    # paired Wd loads halve the per-queue dispatch count
                    wd_sb = wdpool.tile([P, 2, NB], mdt, tag="wd")
                    eng = nc.sync if fc2 % 2 == 0 else nc.scalar
                    eng.dma_start(
                        out=wd_sb,
                        in_=wd_d[2 * fc2 : 2 * fc2 + 2, :, ds].rearrange(
                            "f p d -> p f d"
                        ),
                    )
                    for fi in range(2):
                        fc = 2 * fc2 + fi
                        for j in range(mgl):
                            mt = mg0 + j
                            pl = min(P, tch - mt * P)
                            nc.tensor.matmul(
                                ps_os[j][:pl, :],
                                lhsT=h_all[:, fc, mt * P : mt * P + pl],
                                rhs=wd_sb[:, fi, :],
                                start=(fc == 0),
                                stop=(fc == FM - 1),
                            )
                for j in range(mgl):
                    mt = mg0 + j
                    pl = min(P, tch - mt * P)
                    o_sb = opool.tile([P, NB], F32, tag="o")
                    # alternate the w[t] scaling between DVE and the otherwise
                    # idle ACT engine so group drains aren't serialized on DVE
                    if j % 2 == 0:
                        nc.vector.tensor_scalar_mul(
                            o_sb[:pl, :], ps_os[j][:pl, :], w_sb[:pl, mt : mt + 1]
                        )
                    else:
                        nc.scalar.activation(
                            out=o_sb[:pl, :],
                            in_=ps_os[j][:pl, :],
                            func=mybir.ActivationFunctionType.Copy,
                            scale=w_sb[:pl, mt : mt + 1],
                        )
                    nc.gpsimd.dma_start(
                        out=out_d[t0 + mt * P : t0 + mt * P + pl, ds], in_=o_sb[:pl, :]
                    )


_NC_CACHE: dict = {}


def _get_nc(Tc: int):
    if Tc not in _NC_CACHE:
        _NC_CACHE[Tc] = _build_nc(Tc)
    return _NC_CACHE[Tc]


def _round_tf32(a):
    """Round-to-nearest-even fp32 -> TF32 (10-bit mantissa), as np.float32."""
    u = a.astype(np.float32).view(np.uint32).astype(np.uint64)
    lsb = (u >> 13) & 1
    r = (u + 0x0FFF + lsb) & 0xFFFFE000
    return r.astype(np.uint32).view(np.float32)


def _mm_round(a):
    """Convert a host array to the dtype/value the device matmuls consume."""
    if MM_DT is F32R:
        return _round_tf32(a)
    if MM_DT is F16:
        return np.ascontiguousarray(a, dtype=np.float16)
    return np.ascontiguousarray(a, dtype=np.float32)


def _route_host(x, Wr):
    """Reference-identical routing on host (jax on CPU, same ops as reference).

    Returns (k_ids [T, K] int, k_w [T, K] f32).
    """
    import jax
    import jax.numpy as jnp

    cpu = jax.devices("cpu")[0]
    with jax.default_device(cpu):
        xt = jnp.asarray(x.reshape(T, D))
        logits = jnp.einsum("td,ed->te", xt, jnp.asarray(Wr))
        scores = jax.nn.softmax(logits, axis=-1)
        k_scores, k_ids = jax.lax.top_k(scores, TOPK)
        eps = jnp.finfo(x.dtype).eps
        k_w = k_scores / (k_scores.sum(axis=-1, keepdims=True) + eps)
        return np.asarray(k_ids), np.asarray(k_w)


def _prep_weights(Wg, Wu, Wd):
    """Per-expert weight tensors in device layouts (contiguous f32, rounded)."""
    wg_r, wu_r, wd_r = [], [], []
    for e in range(len(Wg)):
        # Wg[e]: [DFF, D]; device wants [fm, p(d_inner), dc, f_inner]
        wgt = Wg[e].T.reshape(DC, P, FM, P).transpose(2, 1, 0, 3)
        wut = Wu[e].T.reshape(DC, P, FM, P).transpose(2, 1, 0, 3)
        # Wd[e]: [D, DFF]; device wants WdT = [fc, p(f_inner), d]
        wdt = Wd[e].T.reshape(FM, P, D)
        wg_r.append(_mm_round(np.ascontiguousarray(wgt, dtype=np.float32)))
        wu_r.append(_mm_round(np.ascontiguousarray(wut, dtype=np.float32)))
        wd_r.append(_mm_round(np.ascontiguousarray(wdt, dtype=np.float32)))
    return wg_r, wu_r, wd_r


def kernel(x, Wr, Wg, Wu, Wd):
    global LAST_EXEC_NS
    x = np.asarray(x, dtype=np.float32)
    Wr = np.asarray(Wr, dtype=np.float32)
    Wg = np.asarray(Wg, dtype=np.float32)
    Wu = np.asarray(Wu, dtype=np.float32)
    Wd = np.asarray(Wd, dtype=np.float32)

    k_ids, k_w = _route_host(x, Wr)
    xt = x.reshape(T, D)

    # Gather per-expert token lists (each token appears once per selected expert).
    idx_lists, w_lists = [], []
    for e in range(E):
        tmask = k_ids == e                       # [T, K]
        tok = np.nonzero(tmask.any(axis=1))[0]   # unique tokens routed to e
        wvals = (k_w * tmask).sum(axis=1)[tok].astype(np.float32)
        idx_lists.append(tok)
        w_lists.append(wvals)

    maxload = max(len(t) for t in idx_lists)
    # Exact capacity rounded to 8 tokens (16B-aligned fp16 DMA runs); the last
    # matmul token-tile is partial (M < 128) rather than zero-padded to 128.
    Tc = max(P, ((maxload + 7) // 8) * 8)

    wg_r, wu_r, wd_r = _prep_weights(Wg, Wu, Wd)

    in_maps = []
    for e in range(E):
        tok = idx_lists[e]
        xg = np.zeros((Tc, D), dtype=np.float32)
        xg[: len(tok)] = xt[tok]
        # device layout [p(d_inner), dc, t]
        xg_r = np.ascontiguousarray(
            xg.T.reshape(DC, P, Tc).transpose(1, 0, 2), dtype=np.float32
        )
        ntile_all = math.ceil(Tc / P)
        wv = np.zeros((ntile_all * P,), dtype=np.float32)
        wv[: len(tok)] = w_lists[e]
        in_maps.append(
            {
                "x": _mm_round(xg_r),
                "w": np.ascontiguousarray(wv.reshape(ntile_all, P)),
                "wg": wg_r[e],
                "wu": wu_r[e],
                "wd": wd_r[e],
            }
        )

    nc = _get_nc(Tc)
    core_ids = list(range(NCORES))
    if PROFILE:
        res = _run_profiled(nc, in_maps, core_ids)
        LAST_EXEC_NS = res.exec_time_ns
        results = res.results
    else:
        results = run_bass_kernel_spmd(nc, in_maps, core_ids).results

    out = np.zeros((T, D), dtype=np.float32)
    for e in range(E):
        tok = idx_lists[e]
        out[tok] += results[e]["out"][: len(tok)]
    return out.reshape(B, L, D)


def _run_profiled(nc, in_maps, core_ids):
    """run_bass_kernel_spmd with trace=True, providing the NTFF hook that the
    agent image's antenv stub lacks, and skipping the artifact upload."""
    import sys
    import tempfile
    import types

    import concourse.bass_utils as bu

    if "antenv.axon_hooks" not in sys.modules:
        from trn_agent_boot.trn_boot import _ntff_profile_via_ctypes

        hook = _ntff_profile_via_ctypes("/opt/axon/libaxon_pjrt.so")
        mod = types.ModuleType("antenv.axon_hooks")
        mod.get_axon_ntff_profile_hook = lambda: hook
        mod.set_axon_ntff_profile_hook = lambda h: None
        sys.modules["antenv.axon_hooks"] = mod

    orig_upload = bu.upload_artifacts
    bu.upload_artifacts = lambda tmpdir: ""
    try:
        return run_bass_kernel_spmd(
            nc,
            in_maps,
            core_ids,
            trace=True,
            trace_cores=TRACE_CORES,
            tmpdir=tempfile.mkdtemp(prefix="moe_ntff_"),
        )
    finally:
        bu.upload_artifacts = orig_upload


if __name__ == "__main__":
    # smoke test with random data (no reference comparison)
    rng = np.random.default_rng(0)
    ins = {
        "x": rng.standard_normal((B, L, D), dtype=np.float32),
        "Wr": (rng.standard_normal((E, D)) * 0.02).astype(np.float32),
        "Wg": (rng.standard_normal((E, DFF, D)) * 0.02).astype(np.float32),
        "Wu": (rng.standard_normal((E, DFF, D)) * 0.02).astype(np.float32),
        "Wd": (rng.standard_normal((E, D, DFF)) * 0.02).astype(np.float32),
    }
    out = kernel(**ins)
    print("out", out.shape, out.dtype, float(np.abs(out).max()))



# revision 5
# speedup vs baseline: 1.0717x; 1.0717x over previous
"""MoE (top-2 routed GluMLP) Trainium2 kernel, DFF-sharded over 8 NeuronCores.

Contract: kernel(**inputs) takes the FULL unsharded inputs
  x  [2, 2048, 1024] f32
  Wr [8, 1024] f32           router
  Wg [8, 4096, 1024] f32     gate proj per expert
  Wu [8, 4096, 1024] f32     up proj per expert
  Wd [8, 1024, 4096] f32     down proj per expert
and returns the FULL output [2, 2048, 1024] f32.

Strategy (DFF-parallel, perfectly load-balanced):
  - Routing (softmax + top-2 + renormalize) on host with jax on CPU using the
    exact reference ops, so selected experts / combine weights match exactly.
  - The 2*T = 8192 (token, expert) pairs are sorted by expert and processed by
    EVERY core, but each core only computes a 512-wide slice of DFF (4096/8).
    Work per core is identical regardless of routing -> no capacity padding.
  - Pairs are grouped into per-expert "blocks" of <=512 tokens (balanced sizes,
    >=344 here) so every matmul free dim is large and single-expert.
  - Per block: phase B computes h = relu(x@WgT) * (x@WuT) for the f-slice
    (fused in one DVE scalar_tensor_tensor op), then phase C immediately
    computes the partial down projection with Wd^T tiles stationary and h
    moving, writing fp16 partials [d, tokens] to DRAM. Tensor engine never
    waits between phases.
  - Host sums the 8 per-core partials, applies the combine weights, and
    scatter-adds into the final output.
  - Matmul operands fp16 (same 10-bit mantissa as TF32), fp32 PSUM accumulate:
    ~5e-4 rel err. (fp8 measured >=2.7e-2 in simulation - over the 2e-2 gate.)
"""

import math
from contextlib import ExitStack

import numpy as np

import concourse.bass as bass
import concourse.tile as tile
from concourse import bacc, mybir
from concourse.bass_utils import run_bass_kernel_spmd

B, L, D, E, TOPK, DFF = 2, 2048, 1024, 8, 2, 4096
T = B * L
PAIRS = TOPK * T        # 8192 (token, expert) pairs, all cores see all pairs
NCORES = 8
P = 128
NBT = 512               # max moving-block (one fp32 PSUM bank)
DC = D // P             # 8 contraction chunks over D
FMC = DFF // NCORES // P  # 4 f-tiles per core (512-wide DFF slice)
DT = D // P             # 8 output d-tiles
FSL = FMC * P           # 512 f per core

F32 = mybir.dt.float32
F16 = mybir.dt.float16
ALU = mybir.AluOpType

PROFILE = False
TRACE_CORES = None
LAST_EXEC_NS = None
N_WARM = 12             # warm-up matmuls to lift the HAM clock gate during DMA ramp


def _make_blocks(loads):
    """Per-expert balanced blocks of <=NBT pairs: list of (g0, ln, e)."""
    blocks, g = [], 0
    for e, ld in enumerate(loads):
        if ld == 0:
            continue
        nb = math.ceil(ld / NBT)
        base, rem = divmod(ld, nb)
        for i in range(nb):
            ln = base + (1 if i < rem else 0)
            blocks.append((g, ln, e))
            g += ln
    assert g == sum(loads)
    return blocks


def _build_nc(blocks):
    nc = bacc.Bacc(
        "TRN2",
        target_bir_lowering=False,
        debug=False,
        enable_asserts=False,
        num_devices=NCORES,
    )
    xg_d = nc.dram_tensor("xg", [P, DC, PAIRS], F16, kind="ExternalInput").ap()
    wg_d = nc.dram_tensor("wg", [E, FMC, P, DC, P], F16, kind="ExternalInput").ap()
    wu_d = nc.dram_tensor("wu", [E, FMC, P, DC, P], F16, kind="ExternalInput").ap()
    wdt_d = nc.dram_tensor("wdt", [E, P, FMC, D], F16, kind="ExternalInput").ap()
    out_d = nc.dram_tensor("out", [DT, P, PAIRS], F16, kind="ExternalOutput").ap()

    with tile.TileContext(nc) as tc:
        with ExitStack() as ctx:
            _moe_body(ctx, tc, xg_d, wg_d, wu_d, wdt_d, out_d, blocks)
    nc.compile()
    return nc


def _moe_body(ctx, tc, xg_d, wg_d, wu_d, wdt_d, out_d, blocks):
    nc = tc.nc

    xpool = ctx.enter_context(tc.tile_pool(name="xpool", bufs=4))
    hpool = ctx.enter_context(tc.tile_pool(name="hpool", bufs=3))
    wgpool = ctx.enter_context(tc.tile_pool(name="wgpool", bufs=2))
    wupool = ctx.enter_context(tc.tile_pool(name="wupool", bufs=2))
    wdtpool = ctx.enter_context(tc.tile_pool(name="wdtpool", bufs=2))
    gpool = ctx.enter_context(tc.tile_pool(name="gpool", bufs=3))
    opool = ctx.enter_context(tc.tile_pool(name="opool", bufs=12))
    warmpool = ctx.enter_context(tc.tile_pool(name="warmpool", bufs=1))
    psP = ctx.enter_context(tc.tile_pool(name="psP", bufs=8, space="PSUM"))

    # Warm-up: keep the PE busy while the first DMAs land so the HAM clock
    # gate (4/8 cold -> 8/8 warm after ~3.4us of activity) flips before the
    # real matmuls start.
    if N_WARM:
        warm = warmpool.tile([P, P], F16, tag="warm")
        nc.vector.memset(warm, 0.0)
        ps_w = psP.tile([P, NBT], F32, tag="ps", name="ps_warm")
        for _ in range(N_WARM):
            nc.tensor.matmul(ps_w[:, :P], lhsT=warm, rhs=warm, start=True, stop=True)
        # dummy reader so the verifier sees the warm bank consumed
        nc.vector.tensor_copy(out=warm, in_=ps_w[:, :P])

    # x blocks stream with 2-block lookahead, striped over the three DMA queues
    x_tiles = {}

    def issue_x(bi):
        if bi >= len(blocks):
            return
        g0, ln, _ = blocks[bi]
        xt_ = xpool.tile([P, DC, NBT], F16, tag="x", name=f"x{bi}")
        engs = (nc.sync, nc.scalar, nc.gpsimd)
        for dc in range(DC):
            engs[dc % 3].dma_start(out=xt_[:, dc, :ln], in_=xg_d[:, dc, g0 : g0 + ln])
        x_tiles[bi] = xt_

    # expert weight tiles, loaded one expert ahead
    wg_sb, wu_sb, wdt_sb = {}, {}, {}

    def load_expert(e):
        if e is None or e in wg_sb:
            return
        wg_t = wgpool.tile([P, FMC, DC, P], F16, tag="wg", name=f"wg{e}")
        wu_t = wupool.tile([P, FMC, DC, P], F16, tag="wu", name=f"wu{e}")
        for fm in range(FMC):
            nc.sync.dma_start(out=wg_t[:, fm], in_=wg_d[e, fm])
            nc.scalar.dma_start(out=wu_t[:, fm], in_=wu_d[e, fm])
        wd_t = wdtpool.tile([P, FMC, D], F16, tag="wdt", name=f"wdt{e}")
        nc.gpsimd.dma_start(out=wd_t, in_=wdt_d[e])
        wg_sb[e], wu_sb[e], wdt_sb[e] = wg_t, wu_t, wd_t

    eseq = []
    for _, _, e in blocks:
        if not eseq or eseq[-1] != e:
            eseq.append(e)
    enext = {e: (eseq[i + 1] if i + 1 < len(eseq) else None) for i, e in enumerate(eseq)}
    first_block_of = {}
    for bi, (_, _, e) in enumerate(blocks):
        first_block_of.setdefault(e, bi)

    issue_x(0)
    load_expert(eseq[0])
    issue_x(1)
    load_expert(enext[eseq[0]])

    out_engs = (nc.gpsimd, nc.sync, nc.gpsimd, nc.scalar)
    for bi, (g0, ln, e) in enumerate(blocks):
        issue_x(bi + 2)
        if bi == first_block_of[e]:
            load_expert(enext[e])
        x_sb = x_tiles.pop(bi)
        h_sb = hpool.tile([P, FMC, NBT], F16, tag="h", name=f"h{bi}")

        # Phase B: h[f, t] = relu(x@WgT) * (x@WuT) for this core's f-slice
        for fm in range(FMC):
            ps_g = psP.tile([P, NBT], F32, tag="ps", name="ps_g")
            ps_u = psP.tile([P, NBT], F32, tag="ps", name="ps_u")
            for dc in range(DC):
                nc.tensor.matmul(
                    ps_g[:, :ln],
                    lhsT=wg_sb[e][:, fm, dc],
                    rhs=x_sb[:, dc, :ln],
                    start=(dc == 0),
                    stop=(dc == DC - 1),
                )
            for dc in range(DC):
                nc.tensor.matmul(
                    ps_u[:, :ln],
                    lhsT=wu_sb[e][:, fm, dc],
                    rhs=x_sb[:, dc, :ln],
                    start=(dc == 0),
                    stop=(dc == DC - 1),
                )
            g_sb = gpool.tile([P, NBT], F32, tag="g", name="g_sb")
            nc.scalar.activation(
                out=g_sb[:, :ln],
                in_=ps_g[:, :ln],
                func=mybir.ActivationFunctionType.Relu,
            )
            nc.vector.tensor_mul(h_sb[:, fm, :ln], g_sb[:, :ln], ps_u[:, :ln])

        # Phase C: out[d, t] += WdT-slice.T @ h  (WdT tiles stationary,
        # tokens moving -> no partial-tile or boundary waste)
        for dt in range(DT):
            ps_o = psP.tile([P, NBT], F32, tag="ps", name="ps_o")
            for fm in range(FMC):
                nc.tensor.matmul(
                    ps_o[:, :ln],
                    lhsT=wdt_sb[e][:, fm, dt * P : (dt + 1) * P],
                    rhs=h_sb[:, fm, :ln],
                    start=(fm == 0),
                    stop=(fm == FMC - 1),
                )
            o_sb = opool.tile([P, NBT], F16, tag="o", name="o_sb")
            if dt % 2 == 0:
                nc.scalar.copy(out=o_sb[:, :ln], in_=ps_o[:, :ln])
            else:
                nc.vector.tensor_copy(out=o_sb[:, :ln], in_=ps_o[:, :ln])
            out_engs[dt % 4].dma_start(
                out=out_d[dt, :, g0 : g0 + ln], in_=o_sb[:, :ln]
            )


_NC_CACHE: dict = {}


def _get_nc(blocks):
    key = tuple(blocks)
    if key not in _NC_CACHE:
        _NC_CACHE[key] = _build_nc(blocks)
    return _NC_CACHE[key]


def _route_host(x, Wr):
    """Reference-identical routing on host (jax on CPU, same ops as reference)."""
    import jax
    import jax.numpy as jnp

    cpu = jax.devices("cpu")[0]
    with jax.default_device(cpu):
        xt = jnp.asarray(x.reshape(T, D))
        logits = jnp.einsum("td,ed->te", xt, jnp.asarray(Wr))
        scores = jax.nn.softmax(logits, axis=-1)
        k_scores, k_ids = jax.lax.top_k(scores, TOPK)
        eps = jnp.finfo(x.dtype).eps
        k_w = k_scores / (k_scores.sum(axis=-1, keepdims=True) + eps)
        return np.asarray(k_ids), np.asarray(k_w)


def kernel(x, Wr, Wg, Wu, Wd):
    global LAST_EXEC_NS
    x = np.asarray(x, dtype=np.float32)
    Wr = np.asarray(Wr, dtype=np.float32)
    Wg = np.asarray(Wg, dtype=np.float32)
    Wu = np.asarray(Wu, dtype=np.float32)
    Wd = np.asarray(Wd, dtype=np.float32)

    k_ids, k_w = _route_host(x, Wr)
    xt = x.reshape(T, D)

    tok_l, w_l, loads = [], [], []
    for e in range(E):
        tmask = k_ids == e
        tok = np.nonzero(tmask.any(axis=1))[0]
        wv = (k_w * tmask).sum(axis=1)[tok].astype(np.float32)
        tok_l.append(tok)
        w_l.append(wv)
        loads.append(len(tok))
    assert sum(loads) == PAIRS
    tok_all = np.concatenate(tok_l)
    blocks = _make_blocks(loads)

    # gathered pair inputs, device layout [p(d_inner), dc, pair]
    xg16 = xt[tok_all].astype(np.float16)
    xg_dev = np.ascontiguousarray(xg16.T.reshape(DC, P, PAIRS).transpose(1, 0, 2))

    in_maps = []
    for c in range(NCORES):
        sl = slice(c * FSL, (c + 1) * FSL)
        # Wg/Wu rows f-slice: [E, 512, D] -> [E, FMC, P(d_inner), DC, P(f)]
        wg_c = (
            Wg[:, sl, :]
            .transpose(0, 2, 1)
            .reshape(E, DC, P, FMC, P)
            .transpose(0, 3, 2, 1, 4)
            .astype(np.float16)
        )
        wu_c = (
            Wu[:, sl, :]
            .transpose(0, 2, 1)
            .reshape(E, DC, P, FMC, P)
            .transpose(0, 3, 2, 1, 4)
            .astype(np.float16)
        )
        # WdT f-slice: [E, D, 512] -> [E, P(f_inner), FMC, D]
        wdt_c = (
            Wd[:, :, sl]
            .transpose(0, 2, 1)
            .reshape(E, FMC, P, D)
            .transpose(0, 2, 1, 3)
            .astype(np.float16)
        )
        in_maps.append(
            {
                "xg": xg_dev,
                "wg": np.ascontiguousarray(wg_c),
                "wu": np.ascontiguousarray(wu_c),
                "wdt": np.ascontiguousarray(wdt_c),
            }
        )

    nc = _get_nc(blocks)
    core_ids = list(range(NCORES))
    if PROFILE:
        res = _run_profiled(nc, in_maps, core_ids)
        LAST_EXEC_NS = res.exec_time_ns
        results = res.results
    else:
        results = run_bass_kernel_spmd(nc, in_maps, core_ids).results

    # combine: sum f-slice partials, apply routing weights, scatter-add
    acc = np.zeros((D, PAIRS), dtype=np.float32)
    for c in range(NCORES):
        acc += results[c]["out"].reshape(D, PAIRS).astype(np.float32)
    accT = acc.T  # [PAIRS, D]
    out = np.zeros((T, D), dtype=np.float32)
    p0 = 0
    for e in range(E):
        ln = loads[e]
        out[tok_l[e]] += w_l[e][:, None] * accT[p0 : p0 + ln]
        p0 += ln
    return out.reshape(B, L, D)


def _run_profiled(nc, in_maps, core_ids):
    """run_bass_kernel_spmd with trace=True, providing the NTFF hook that the
    agent image's antenv stub lacks, and skipping the artifact upload."""
    import sys
    import tempfile
    import types

    import concourse.bass_utils as bu

    if "antenv.axon_hooks" not in sys.modules:
        from trn_agent_boot.trn_boot import _ntff_profile_via_ctypes

        hook = _ntff_profile_via_ctypes("/opt/axon/libaxon_pjrt.so")
        mod = types.ModuleType("antenv.axon_hooks")
        mod.get_axon_ntff_profile_hook = lambda: hook
        mod.set_axon_ntff_profile_hook = lambda h: None
        sys.modules["antenv.axon_hooks"] = mod

    orig_upload = bu.upload_artifacts
    bu.upload_artifacts = lambda tmpdir: ""
    try:
        return run_bass_kernel_spmd(
            nc,
            in_maps,
            core_ids,
            trace=True,
            trace_cores=TRACE_CORES,
            tmpdir=tempfile.mkdtemp(prefix="moe_ntff_"),
        )
    finally:
        bu.upload_artifacts = orig_upload


if __name__ == "__main__":
    # smoke test with random data (no reference comparison)
    rng = np.random.default_rng(0)
    ins = {
        "x": rng.standard_normal((B, L, D), dtype=np.float32),
        "Wr": (rng.standard_normal((E, D)) * 0.02).astype(np.float32),
        "Wg": (rng.standard_normal((E, DFF, D)) * 0.02).astype(np.float32),
        "Wu": (rng.standard_normal((E, DFF, D)) * 0.02).astype(np.float32),
        "Wd": (rng.standard_normal((E, D, DFF)) * 0.02).astype(np.float32),
    }
    out = kernel(**ins)
    print("out", out.shape, out.dtype, float(np.abs(out).max()))
